# revision 9
# baseline (speedup 1.0000x reference)
"""HRVQ (3-level residual VQ) Trainium2 kernel — data-parallel over tokens on 8 cores.

Per-core pipeline (natural [token, D] layout, 512-token tiles):
  - PE identity-transposes r -> rT chunks; fp32 matmul scores m = r@C.T - |c|^2/2
  - DVE max_with_indices -> argmin + min distance (loss via dmin identity)
  - q via indirect DMA gather (exact fp32 codebook rows)
  - residual update in-place; EMA sums via bf16 one-hot matmul, counts via ones-matmul
  - one AllReduce (sums/counts/loss) + on-device EMA finalize
"""
import numpy as np

NUM_LEVELS = 3
K = 512
D = 1024
B, T = 16, 4096
N = B * T
NCORES = 8
NPC = N // NCORES          # tokens per core
TT = 512                   # tokens per tile
NSUB = TT // 128           # 128-token sub-tiles per tile
NTILES = NPC // TT
COMMIT = (0.25, 0.5, 1.0)
DECAY = 0.99
EPS = 1e-05

_BUILT = {}


def _install_tilepatch():
    """walrus here rejects >2 sem waits on one instruction; spread the Tile
    tail-drain waits over preceding SP nops (SP executes in order)."""
    import concourse.mybir as mybir
    import concourse.tile as tile_mod
    from concourse.vector_clock import ScopedClock

    MAX_NOPS = 48

    def _patched(self, tick_clock, wait_clock):
        nops = [self.nc.sync.nop(nofuse=True, hint=f"drainwait{i}") for i in range(MAX_NOPS)]
        drain_inst = self.nc.sync.drain()
        wait_clock.add_sem_waits(drain_inst.ins, ScopedClock({None: tick_clock.global_clock}))
        si = drain_inst.ins.sync_info
        waits = list(si.on_wait or [])
        if len(waits) > 1:
            assert len(waits) <= MAX_NOPS + 1, f"{len(waits)} drain waits"
            for i, w in enumerate(waits[:-1]):
                ni = nops[i].ins
                nsi = ni.sync_info
                if nsi is None:
                    ni.sync_info = mybir.SyncInfo(on_wait=[w], on_update=[])
                else:
                    nsi.on_wait = (nsi.on_wait or []) + [w]
            si.on_wait = [waits[-1]]
        self.nc.all_engine_barrier()
        popped = self.nc._tile_sem_poison_stack.pop()
        assert popped is self._sem_poison
        self.nc.clear_and_free_semaphores(list(self.sems.allocated().values()))
        self.nc.all_engine_barrier()

    tile_mod.TileContext._drain_and_barrier = _patched


def _build():
    import concourse.bass as bass
    import concourse.mybir as mybir
    from concourse.tile import TileContext
    from concourse.masks import make_identity

    _install_tilepatch()
    f32 = mybir.dt.float32
    bf16 = mybir.dt.bfloat16
    u32 = mybir.dt.uint32
    i32 = mybir.dt.int32
    Alu = mybir.AluOpType
    Act = mybir.ActivationFunctionType

    nc = bass.Bass()
    z_in = nc.declare_dram_parameter("z", [NPC, D], f32, isOutput=False)
    cb_in = nc.declare_dram_parameter("cb", [NUM_LEVELS * K, D], f32, isOutput=False)
    ct_in = nc.declare_dram_parameter("ct", [NUM_LEVELS, D, K], f32, isOutput=False)
    cnh_in = nc.declare_dram_parameter("cnh", [NUM_LEVELS, 128, K], f32, isOutput=False)
    emac_in = nc.declare_dram_parameter("emac", [NUM_LEVELS, K], f32, isOutput=False)
    emaw_in = nc.declare_dram_parameter("emaw", [NUM_LEVELS * K, D], f32, isOutput=False)

    zq_out = nc.declare_dram_parameter("zq", [NPC, D], f32, isOutput=True)
    idx_out = nc.declare_dram_parameter("idx", [NUM_LEVELS, NPC], i32, isOutput=True)
    ncb_out = nc.declare_dram_parameter("ncb", [NUM_LEVELS * K, D], f32, isOutput=True)
    loss_out = nc.declare_dram_parameter("loss", [1, 1], f32, isOutput=True)

    RC = NUM_LEVELS * K          # 1536 rows of sums in the collective buffer
    cc_in = nc.dram_tensor("cc_in", [RC + 4, D], f32)
    cc_out = nc.dram_tensor("cc_out", [RC + 4, D], f32, addr_space="Shared")
    csn_dram = nc.dram_tensor("csn_dram", [NUM_LEVELS, K], f32)

    with TileContext(nc) as tc:
        with tc.tile_pool(name="consts", bufs=1) as consts:
            ident = consts.tile([128, 128], f32)
            make_identity(nc, ident)
            iota_i = consts.tile([128, K], u32)
            nc.gpsimd.iota(iota_i[:], pattern=[[1, K]], base=0, channel_multiplier=0)
            iota_f = consts.tile([128, K], f32)
            nc.vector.tensor_copy(iota_f[:], iota_i[:])
            ones_bf = consts.tile([128, 1], bf16)
            nc.vector.memset(ones_bf[:], 1.0)
            ones_f = consts.tile([128, 1], f32)
            nc.vector.memset(ones_f[:], 1.0)

            ct = consts.tile([128, NUM_LEVELS, 8, K], f32)   # ct[:, l, c, :] = C_l.T rows 128c..
            nc.sync.dma_start(out=ct[:], in_=ct_in[:].rearrange("l (c p) k -> p l c k", p=128))
            cnh = consts.tile([128, NUM_LEVELS, K], f32)
            nc.sync.dma_start(out=cnh[:], in_=cnh_in[:].rearrange("l p k -> p l k"))

            # persistent accumulators
            sums_acc = consts.tile([128, NUM_LEVELS, 4, D], bf16)
            nc.vector.memset(sums_acc[:], 0.0)
            counts_acc = consts.tile([1, NUM_LEVELS, K], f32)
            nc.vector.memset(counts_acc[:], 0.0)
            loss_acc = consts.tile([128, 1], f32)
            nc.vector.memset(loss_acc[:], 0.0)

            with (
                tc.tile_pool(name="work", bufs=1) as wp,
                tc.tile_pool(name="io", bufs=2) as iop,
                tc.tile_pool(name="ps", bufs=1, space="PSUM") as pp,
            ):
                for ti in range(NTILES):
                    z = iop.tile([128, NSUB, D], f32, tag="z")
                    nc.sync.dma_start(out=z[:], in_=z_in[:].rearrange("(t s p) d -> p t s d", p=128, s=NSUB)[:, ti])
                    r = wp.tile([128, NSUB, D], f32, tag="r")
                    r16 = wp.tile([128, NSUB, D], bf16, tag="r16")
                    q = iop.tile([128, NSUB, D], f32, tag="q")
                    oh = wp.tile([128, NSUB, K], bf16, tag="oh")
                    rsq = wp.tile([128, NSUB], f32, tag="rsq")
                    idxst = wp.tile([128, NUM_LEVELS, NSUB], u32, tag="idxst")
                    sqd = wp.tile([128, D], f32, tag="sqd")
                    for s in range(NSUB):
                        nc.scalar.activation(sqd[:], z[:, s, :], Act.Square,
                                             accum_out=rsq[:, s:s + 1])

                    for l in range(NUM_LEVELS):
                        rin = z if l == 0 else r
                        # bf16 copy of residual for the EMA-sums matmul
                        for s in range(NSUB):
                            nc.scalar.copy(r16[:, s, :], rin[:, s, :])
                        for s in range(NSUB):
                            # rT chunks via PE transpose
                            tp = pp.tile([128, 8, 128], f32, tag="tp")
                            for c in range(8):
                                nc.tensor.transpose(tp[:, c, :], rin[:, s, 128 * c:128 * (c + 1)], ident[:])
                            rt = wp.tile([128, 8, 128], f32, tag="rt")
                            nc.scalar.copy(rt[:], tp[:])
                            # scores
                            sc = pp.tile([128, K], f32, tag="sc")
                            for c in range(8):
                                nc.tensor.matmul(sc[:], rt[:, c, :], ct[:, l, c, :],
                                                 start=(c == 0), stop=(c == 7))
                            m = wp.tile([128, K], f32, tag="m")
                            nc.vector.tensor_sub(m[:], sc[:], cnh[:, l, :])
                            mx = wp.tile([128, 8], f32, tag="mx")
                            mi = wp.tile([128, 8], u32, tag="mi")
                            nc.vector.max_with_indices(mx[:], mi[:], m[:])
                            nc.vector.tensor_copy(idxst[:, l, s:s + 1], mi[:, 0:1])
                            # dmin_t = |r|^2 - 2*max -> becomes |r_next|^2
                            nc.vector.scalar_tensor_tensor(
                                out=rsq[:, s:s + 1], in0=mx[:, 0:1], scalar=-2.0,
                                in1=rsq[:, s:s + 1], op0=Alu.mult, op1=Alu.add)
                            nc.vector.scalar_tensor_tensor(
                                out=loss_acc[:], in0=rsq[:, s:s + 1], scalar=float(COMMIT[l]),
                                in1=loss_acc[:], op0=Alu.mult, op1=Alu.add)
                            # one-hot (bf16) from index
                            mif = wp.tile([128, 1], f32, tag="mif")
                            nc.vector.tensor_copy(mif[:], mi[:, 0:1])
                            nc.vector.tensor_scalar(out=oh[:, s, :], in0=iota_f[:], scalar1=mif[:, 0:1],
                                                    scalar2=None, op0=Alu.is_equal)
                            # gather q = C_l[idx]
                            ioff = wp.tile([128, 1], u32, tag="ioff")
                            nc.vector.tensor_scalar(out=ioff[:], in0=mi[:, 0:1], scalar1=l * K,
                                                    scalar2=None, op0=Alu.add)
                            nc.gpsimd.indirect_dma_start(
                                out=q[:, s, :], out_offset=None, in_=cb_in[:],
                                in_offset=bass.IndirectOffsetOnAxis(ap=ioff[:, 0:1], axis=0))
                            # residual update (in-place for l>0)
                            nc.vector.scalar_tensor_tensor(
                                out=r[:, s, :], in0=q[:, s, :], scalar=-1.0,
                                in1=rin[:, s, :], op0=Alu.mult, op1=Alu.add)

                        # EMA stats: sums_l += onehot^T @ r16 ; counts_l += onehot col-sums
                        smp = pp.tile([128, 4, 512], f32, tag="smp")
                        for h in range(2):
                            for kc in range(4):
                                for s in range(NSUB):
                                    nc.tensor.matmul(
                                        smp[:, kc, :],
                                        oh[:, s, 128 * kc:128 * (kc + 1)],
                                        r16[:, s, 512 * h:512 * (h + 1)],
                                        start=(s == 0), stop=(s == NSUB - 1))
                            for kc in range(4):
                                nc.vector.tensor_add(
                                    out=sums_acc[:, l, kc, 512 * h:512 * (h + 1)],
                                    in0=sums_acc[:, l, kc, 512 * h:512 * (h + 1)],
                                    in1=smp[:, kc, :])
                        ctp = pp.tile([1, K], f32, tag="ctp")
                        for s in range(NSUB):
                            nc.tensor.matmul(ctp[:], ones_bf[:], oh[:, s, :],
                                             start=(s == 0), stop=(s == NSUB - 1))
                        nc.vector.tensor_add(out=counts_acc[:, l, :], in0=counts_acc[:, l, :], in1=ctp[:])

                    # z_q = z - r3 (exact); stage in q buffer then store
                    for s in range(NSUB):
                        nc.vector.scalar_tensor_tensor(
                            out=q[:, s, :], in0=r[:, s, :], scalar=-1.0,
                            in1=z[:, s, :], op0=Alu.mult, op1=Alu.add)
                    nc.sync.dma_start(out=zq_out[:].rearrange("(t s p) d -> p t s d", p=128, s=NSUB)[:, ti], in_=q[:])
                    for l in range(NUM_LEVELS):
                        nc.sync.dma_start(
                            out=idx_out[:].rearrange("l (t s p) -> p l t s", p=128, s=NSUB)[:, l, ti].bitcast(u32),
                            in_=idxst[:, l, :])

            # ---- epilogue: reduce, all-reduce, EMA finalize ----
            with (
                tc.tile_pool(name="fin", bufs=2) as fp,
                tc.tile_pool(name="fps", bufs=1, space="PSUM") as fpp,
            ):
                lsp = fpp.tile([1, 1], f32, tag="lsp")
                nc.tensor.matmul(lsp[:], loss_acc[:], ones_f[:], start=True, stop=True)
                lsb = fp.tile([1, 1], f32, tag="lsb")
                nc.vector.tensor_copy(lsb[:], lsp[:])
                nc.sync.dma_start(out=cc_in[RC + 3:RC + 4, 0:1], in_=lsb[:])
                # sums (bf16 -> f32) and counts into the collective buffer
                for l in range(NUM_LEVELS):
                    for kc in range(4):
                        sf = fp.tile([128, D], f32, tag="sf")
                        nc.vector.tensor_copy(sf[:], sums_acc[:, l, kc, :])
                        nc.sync.dma_start(out=cc_in[l * K + 128 * kc: l * K + 128 * (kc + 1), :], in_=sf[:])
                    nc.sync.dma_start(out=cc_in[RC + l:RC + l + 1, 0:K], in_=counts_acc[:, l, :])
                nc.gpsimd.collective_compute(
                    "AllReduce", mybir.AluOpType.add,
                    replica_groups=[list(range(NCORES))],
                    ins=[cc_in[:]], outs=[cc_out[:]])
                # cluster-size smoothing
                cnt = fp.tile([NUM_LEVELS, K], f32, tag="cnt")
                nc.sync.dma_start(out=cnt[:], in_=cc_out[RC:RC + NUM_LEVELS, 0:K])
                emac = fp.tile([NUM_LEVELS, K], f32, tag="emac")
                nc.sync.dma_start(out=emac[:], in_=emac_in[:])
                cs = fp.tile([NUM_LEVELS, K], f32, tag="cs")
                nc.vector.tensor_scalar(out=cs[:], in0=emac[:], scalar1=DECAY, scalar2=None, op0=Alu.mult)
                nc.vector.scalar_tensor_tensor(out=cs[:], in0=cnt[:], scalar=1.0 - DECAY,
                                               in1=cs[:], op0=Alu.mult, op1=Alu.add)
                nsum = fp.tile([NUM_LEVELS, 1], f32, tag="nsum")
                nc.vector.tensor_reduce(out=nsum[:], in_=cs[:], axis=mybir.AxisListType.X, op=Alu.add)
                dn = fp.tile([NUM_LEVELS, 1], f32, tag="dn")
                nc.vector.tensor_scalar(out=dn[:], in0=nsum[:], scalar1=float(K * EPS), scalar2=None, op0=Alu.add)
                rec = fp.tile([NUM_LEVELS, 1], f32, tag="rec")
                nc.vector.reciprocal(rec[:], dn[:])
                fac = fp.tile([NUM_LEVELS, 1], f32, tag="fac")
                nc.vector.tensor_tensor(out=fac[:], in0=nsum[:], in1=rec[:], op=Alu.mult)
                csn = fp.tile([NUM_LEVELS, K], f32, tag="csn")
                nc.vector.tensor_scalar(out=csn[:], in0=cs[:], scalar1=EPS, scalar2=None, op0=Alu.add)
                nc.vector.tensor_scalar(out=csn[:], in0=csn[:], scalar1=fac[:, 0:1], scalar2=None, op0=Alu.mult)
                nc.sync.dma_start(out=csn_dram[:], in_=csn[:])
                csn12 = fp.tile([128, NUM_LEVELS, 4], f32, tag="csn12")
                nc.sync.dma_start(out=csn12[:], in_=csn_dram[:].rearrange("l (c p) -> p l c", p=128))
                inv12 = fp.tile([128, NUM_LEVELS, 4], f32, tag="inv12")
                nc.vector.reciprocal(inv12[:], csn12[:])
                nc.vector.tensor_scalar(out=inv12[:], in0=inv12[:], scalar1=1.0 - DECAY,
                                        scalar2=None, op0=Alu.mult)
                # new_codebooks = (DECAY*ema_w + (1-DECAY)*sums) / csn
                for l in range(NUM_LEVELS):
                    for kc in range(4):
                        ew = fp.tile([128, D], f32, tag="ew")
                        nc.sync.dma_start(out=ew[:], in_=emaw_in[l * K + 128 * kc: l * K + 128 * (kc + 1), :])
                        sr = fp.tile([128, D], f32, tag="sr")
                        nc.sync.dma_start(out=sr[:], in_=cc_out[l * K + 128 * kc: l * K + 128 * (kc + 1), :])
                        t1 = fp.tile([128, D], f32, tag="t1")
                        nc.vector.tensor_scalar(out=t1[:], in0=ew[:], scalar1=DECAY / (1.0 - DECAY),
                                                scalar2=None, op0=Alu.mult)
                        nc.vector.tensor_add(out=t1[:], in0=t1[:], in1=sr[:])
                        nc.vector.tensor_scalar(out=t1[:], in0=t1[:], scalar1=inv12[:, l, kc:kc + 1],
                                                scalar2=None, op0=Alu.mult)
                        nc.sync.dma_start(out=ncb_out[l * K + 128 * kc: l * K + 128 * (kc + 1), :], in_=t1[:])
                # loss = allreduced dmin sum / (N*D) with commit weights already applied
                lf = fp.tile([1, 1], f32, tag="lf")
                nc.sync.dma_start(out=lf[:], in_=cc_out[RC + 3:RC + 4, 0:1])
                nc.vector.tensor_scalar(out=lf[:], in0=lf[:], scalar1=1.0 / (N * D), scalar2=None, op0=Alu.mult)
                nc.sync.dma_start(out=loss_out[:], in_=lf[:])
    _legalize_waits(nc, mybir)
    return nc


def _legalize_waits(nc, mybir, max_waits=1):
    """walrus limit: <=2 sem waits per instruction. Hoist excess waits onto
    same-engine nops inserted immediately before the offending instruction."""
    ctr = [0]
    for bb in nc.m.functions[0].blocks:
        il = list(bb.instructions)
        out = []
        changed = False
        for inst in il:
            si = inst.sync_info
            waits = list(si.on_wait) if (si is not None and si.on_wait) else []
            if len(waits) > max_waits:
                changed = True
                excess, keep = waits[:-max_waits], waits[-max_waits:]
                for j in range(0, len(excess), max_waits):
                    ctr[0] += 1
                    nop = mybir.InstNoOp(name=f"lwait{ctr[0]}", ins=[], outs=[])
                    nop.engine = inst.engine
                    nop.sync_info = mybir.SyncInfo(on_wait=excess[j:j + max_waits], on_update=[])
                    out.append(nop)
                si.on_wait = keep
            out.append(inst)
        if changed:
            bb.instructions = out


def kernel(z_e, codebooks, ema_cluster, ema_w):
    from concourse.bass_utils import run_bass_kernel_spmd

    if "nc" not in _BUILT:
        _BUILT["nc"] = _build()
    nc = _BUILT["nc"]

    z_e = np.asarray(z_e, dtype=np.float32)
    codebooks = np.asarray(codebooks, dtype=np.float32)
    ema_cluster = np.asarray(ema_cluster, dtype=np.float32)
    ema_w = np.asarray(ema_w, dtype=np.float32)

    flat = z_e.reshape(N, D)
    ct = np.ascontiguousarray(codebooks.transpose(0, 2, 1))                      # [L, D, K]
    cnh = np.broadcast_to(((codebooks * codebooks).sum(-1) / 2.0)[:, None, :],
                          (NUM_LEVELS, 128, K)).copy().astype(np.float32)        # [L, 128, K]
    cb = codebooks.reshape(NUM_LEVELS * K, D)
    emaw = ema_w.reshape(NUM_LEVELS * K, D)

    in_maps = []
    for c in range(NCORES):
        in_maps.append({
            "z": np.ascontiguousarray(flat[c * NPC:(c + 1) * NPC]),
            "cb": cb, "ct": ct, "cnh": cnh,
            "emac": ema_cluster, "emaw": emaw,
        })
    res = run_bass_kernel_spmd(nc, in_maps, list(range(NCORES)))
    _BUILT["last_res"] = res
    _BUILT["in_maps"] = in_maps
    outs = res.results
    z_q = np.concatenate([outs[c]["zq"] for c in range(NCORES)], axis=0).reshape(B, T, D)
    indices = np.concatenate([outs[c]["idx"] for c in range(NCORES)], axis=1).reshape(NUM_LEVELS, B, T)
    loss = np.float32(outs[0]["loss"][0, 0])
    new_codebooks = outs[0]["ncb"].reshape(NUM_LEVELS, K, D)
    return z_q, loss, indices, new_codebooks
